# revision 21
# baseline (speedup 1.0000x reference)
"""Trainium2 Bass kernel for nn_AlignmentLoss (triplet + CE over phrase/input embeddings).

Sharding: batch dimension N=128 split 16 batches/core across 8 cores.  Each core
owns the positive pairs whose batch_idxs falls in its range (host buckets pairs,
padded to a fixed per-batch capacity).  All cosine distances are entries of the
similarity rows, so per pair the device computes: the sim row against its batch's
K inputs (PE matmul), top-8 row values (single DVE max8 instruction), s_pos and
the two random-negative sims (fused multiply+accumulate dots), and a CE
log-sum-exp (PE matmul + fused Exp/accum).  Hard-negative top-3 with the positive
masked out is recovered from unmasked top-4 values via
    sum_masked = sum_{i<=4} relu(t_i - s_pos + 1) - max(1, relu(t4 - s_pos + 1))
which needs no indexed masking.  Per-core partial sums are summed on host.

Input-row 1/norms: per batch-pair, xt chunks are squared on GpSimd and reduced
over D with 4 accumulating selector matmuls (xtsq as the cheap moving operand)
into a (4, 512) PSUM tile, then a single Abs_reciprocal_sqrt activation writes
1/norm rows; rows-scale broadcasts come from zero-compute-cost 0-stride DMAs.
"""

import sys

for _p in ("/opt/trn_rl_repo", "/root/.axon_site/_ro/trn_rl_repo"):
    if _p not in sys.path:
        sys.path.append(_p)

import numpy as np

import concourse.bass as bass
import concourse.bacc as bacc
import concourse.mybir as mybir
from concourse.tile import TileContext
from concourse.bass_utils import run_bass_kernel_spmd

F32 = mybir.dt.float32
AF = mybir.ActivationFunctionType
ALU = mybir.AluOpType
AX = mybir.AxisListType

N, K, M, D, P = 128, 1024, 512, 128, 4096
NCORES = 8
NB = N // NCORES  # batches per core = 16


def _bcast_parts(ap, nparts):
    """0-stride partition broadcast of a (1, F) AP to (nparts, F) for DMA."""
    return bass.AP(tensor=ap.tensor, offset=ap.offset,
                   ap=[[0, nparts]] + list(ap.ap[1:]))


def _bcast_free(ap, reps):
    """Append a 0-stride innermost free dim: (.., F) -> (.., F, reps)."""
    return bass.AP(tensor=ap.tensor, offset=ap.offset,
                   ap=list(ap.ap) + [[0, reps]])


def build_graph(cap: int, T: float) -> bass.Bass:
    """One-core SPMD graph; cap = padded pairs per batch; T = temperature."""
    C = NB * cap          # padded pairs per core
    NT = C // 128         # 128-pair tiles
    NBP = NB // 2         # batch-pairs (two batches share a 128-partition tile)
    assert cap % 64 == 0 and NT == NBP

    nc = bacc.Bacc(None, target_bir_lowering=False, debug=False)

    xt = nc.declare_dram_parameter("xt", [D, NB * K], F32, isOutput=False)
    phr = nc.declare_dram_parameter("phr", [M, D], F32, isOutput=False)
    anc = nc.declare_dram_parameter("anc", [C, D], F32, isOutput=False)
    pos = nc.declare_dram_parameter("pos", [C, D], F32, isOutput=False)
    rng = nc.declare_dram_parameter("rng", [2 * C, D], F32, isOutput=False)
    vld = nc.declare_dram_parameter("vld", [128, NT], F32, isOutput=False)
    eye = nc.declare_dram_parameter("eye", [128, 128], F32, isOutput=False)
    sel4s = nc.declare_dram_parameter("sel4s", [128, 16], F32, isOutput=False)
    out = nc.declare_dram_parameter("out", [16, 1], F32, isOutput=True)

    with TileContext(nc) as tc:
        with (
            tc.tile_pool(name="big", bufs=1) as big,
            tc.tile_pool(name="chunks", bufs=3) as chunks,
            tc.tile_pool(name="work", bufs=4) as work,
            tc.tile_pool(name="small", bufs=8) as small,
            tc.tile_pool(name="rowsp", bufs=2) as rowsbp,
            tc.tile_pool(name="dram", bufs=1, space="DRAM") as dram,
            tc.tile_pool(name="pn2", bufs=2, space="PSUM") as pn2,
            tc.tile_pool(name="prows", bufs=3, space="PSUM") as prows,
            tc.tile_pool(name="psmall", bufs=2, space="PSUM") as psmall,
        ):
            # ---- constants / small inputs ----
            eye_sb = big.tile([128, 128], F32, tag="eye")
            nc.sync.dma_start(out=eye_sb, in_=eye[:, :])
            sel4s_sb = big.tile([128, 16], F32, tag="sel4s")
            nc.sync.dma_start(out=sel4s_sb, in_=sel4s[:, :])
            vld_sb = big.tile([128, NT], F32, tag="vld")
            nc.sync.dma_start(out=vld_sb, in_=vld[:, :])
            ones_col = big.tile([128, 1], F32, tag="ones")
            nc.vector.memset(ones_col, 1.0)

            # big persistent tensors
            xt_sb = big.tile([128, NB * K], F32, tag="xt")
            anchat = big.tile([128, NT * 128], F32, tag="anchat")
            anchatT = big.tile([128, C], F32, tag="anchatT")
            poshatT = big.tile([128, C], F32, tag="poshatT")
            phatT = big.tile([128, M], F32, tag="phatT")
            rinv32 = big.tile([4, NBP * 512], F32, tag="rinv32")  # [chunk%4, bp*512+k%512]
            rinv_dr = dram.tile([4, NBP * 512], F32, tag="rinvdr")
            t8_all = big.tile([128, NT * 8], F32, tag="t8")
            spos = big.tile([128, NT], F32, tag="spos")
            srnd = big.tile([128, 2 * NT], F32, tag="srnd")
            sumexp = big.tile([128, NT], F32, tag="sumexp")
            stat = big.tile([128, 2 * NT], F32, tag="stat")

            def normalize_tile(dst, src_dram_slice, ptag):
                """Load a (128, D) row block, L2-normalize rows into dst."""
                x = work.tile([128, D], F32, tag=ptag)
                nc.sync.dma_start(out=x, in_=src_dram_slice)
                junk = work.tile([128, D], F32, tag="junk")
                n2c = small.tile([128, 1], F32, tag="n2c")
                nc.scalar.activation(junk, x, AF.Square, accum_out=n2c)
                rin = small.tile([128, 1], F32, tag="rin")
                nc.scalar.activation(rin, n2c, AF.Abs_reciprocal_sqrt)
                nc.vector.tensor_scalar_mul(dst, x, rin)
                return dst

            def transpose_to(dstT_slice, src_tile):
                ps = psmall.tile([128, 512], F32, tag="pm")
                nc.tensor.transpose(ps[:, :128], src_tile, eye_sb)
                nc.vector.tensor_copy(dstT_slice, ps[:, :128])

            # ---- phrases: normalize + transpose -> phatT (D, M) ----
            for t in range(M // 128):
                ph = work.tile([128, D], F32, tag="phn")
                normalize_tile(ph, phr[t * 128:(t + 1) * 128, :], "phx")
                transpose_to(phatT[:, t * 128:(t + 1) * 128], ph)

            # ---- anchors / positives / rand-negs ----
            for t in range(NT):
                a = anchat[:, t * 128:(t + 1) * 128]
                normalize_tile(a, anc[t * 128:(t + 1) * 128, :], "ax")
                transpose_to(anchatT[:, t * 128:(t + 1) * 128], a)
            for t in range(NT):
                po = work.tile([128, D], F32, tag="pon")
                normalize_tile(po, pos[t * 128:(t + 1) * 128, :], "px")
                transpose_to(poshatT[:, t * 128:(t + 1) * 128], po)
                junk2 = work.tile([128, D], F32, tag="junk2")
                nc.vector.scalar_tensor_tensor(
                    junk2, anchat[:, t * 128:(t + 1) * 128], 1.0, po,
                    op0=ALU.mult, op1=ALU.mult,
                    accum_out=spos[:, t:t + 1],
                )
            for r in range(2):
                for t in range(NT):
                    rg = work.tile([128, D], F32, tag="rgn")
                    normalize_tile(
                        rg, rng[r * C + t * 128:r * C + (t + 1) * 128, :], "rx")
                    junk3 = work.tile([128, D], F32, tag="junk3")
                    nc.vector.scalar_tensor_tensor(
                        junk3, anchat[:, t * 128:(t + 1) * 128], 1.0, rg,
                        op0=ALU.mult, op1=ALU.mult,
                        accum_out=srnd[:, 2 * t + r:2 * t + r + 1],
                    )

            # ---- main loop over batch-pairs ----
            for bp in range(NBP):
                # load 4 xt chunks (batches 2bp, 2bp+1); squared-column sums
                # accumulate into row cc of a (4, 512) PSUM tile
                n2w = pn2.tile([4, 512], F32, tag="n2w")
                for cc in range(4):
                    c = 4 * bp + cc
                    sl = xt_sb[:, c * 512:(c + 1) * 512]
                    nc.sync.dma_start(out=sl, in_=xt[:, c * 512:(c + 1) * 512])
                    sq = chunks.tile([128, 512], F32, tag="sq")
                    nc.gpsimd.tensor_mul(sq, sl, sl)
                    nc.tensor.matmul(
                        n2w, sel4s_sb[:, 4 * cc:4 * cc + 4], sq,
                        start=(cc == 0), stop=(cc == 3))
                # rinv rows for the 4 chunks in one fused rsqrt
                nc.scalar.activation(
                    rinv32[:, bp * 512:(bp + 1) * 512], n2w,
                    AF.Abs_reciprocal_sqrt)
                nc.sync.dma_start(
                    out=rinv_dr[:, bp * 512:(bp + 1) * 512],
                    in_=rinv32[:, bp * 512:(bp + 1) * 512])

                # rows-scale broadcast tile via 0-stride DMAs (no compute cost)
                rnb_sb = chunks.tile([128, K], F32, tag="rnbs")
                for h in range(2):          # batch within pair
                    for hh in range(2):     # k half
                        src = rinv_dr[2 * h + hh:2 * h + hh + 1,
                                      bp * 512:(bp + 1) * 512]
                        nc.sync.dma_start(
                            out=rnb_sb[64 * h:64 * h + 64,
                                       hh * 512:(hh + 1) * 512],
                            in_=_bcast_parts(src, 64))

                rows_sb = rowsbp.tile([128, K], F32, tag="rows")
                for h in range(2):
                    rp = prows.tile([128, 512], F32, tag="rp")
                    for half in range(2):
                        b = 2 * bp + half
                        nc.tensor.matmul(
                            rp[half * 64:(half + 1) * 64, :],
                            anchatT[:, b * cap:b * cap + cap],
                            xt_sb[:, b * K + h * 512:b * K + (h + 1) * 512],
                            start=True, stop=True)
                    nc.vector.scalar_tensor_tensor(
                        rows_sb[:, h * 512:(h + 1) * 512], rp, 1.0,
                        rnb_sb[:, h * 512:(h + 1) * 512],
                        op0=ALU.mult, op1=ALU.mult)
                nc.vector.max(t8_all[:, bp * 8:(bp + 1) * 8], rows_sb)

            # ---- CE: logits + exp/accum per pair tile ----
            for t in range(NT):
                lg = psmall.tile([128, 512], F32, tag="pm")
                nc.tensor.matmul(
                    lg, poshatT[:, t * 128:(t + 1) * 128], phatT,
                    start=True, stop=True)
                junk4 = work.tile([128, 512], F32, tag="junk4")
                nc.scalar.activation(
                    junk4, lg, AF.Exp, scale=float(T),
                    accum_out=sumexp[:, t:t + 1])

            # ---- finale, batched over all NT pair tiles ----
            t83 = t8_all[:, :].rearrange("p (t e) -> p t e", e=8)
            u_all = big.tile([128, NT * 8], F32, tag="uall")
            u3 = u_all[:, :].rearrange("p (t e) -> p t e", e=8)
            # u = (t8 + 1) - s_pos
            nc.vector.scalar_tensor_tensor(
                u3, t83, 1.0, _bcast_free(spos[:, :], 8),
                op0=ALU.add, op1=ALU.subtract)
            nc.vector.tensor_scalar_max(u_all, u_all, 0.0)
            s4 = small.tile([128, NT], F32, tag="s4")
            nc.vector.tensor_reduce(s4, u3[:, :, 0:4], AX.X, ALU.add)
            w = small.tile([128, NT], F32, tag="w")
            u4th = u_all[:, 3:4]
            u4th = bass.AP(tensor=u4th.tensor, offset=u4th.offset,
                           ap=[u4th.ap[0], [8, NT]])
            nc.vector.tensor_scalar_max(w, u4th, 1.0)
            hard = small.tile([128, NT], F32, tag="hard")
            nc.vector.tensor_sub(hard, s4, w)
            # random negatives
            ur = small.tile([128, 2 * NT], F32, tag="ur")
            ur3 = ur[:, :].rearrange("p (t e) -> p t e", e=2)
            nc.vector.scalar_tensor_tensor(
                ur3, srnd[:, :].rearrange("p (t e) -> p t e", e=2), 1.0,
                _bcast_free(spos[:, :], 2),
                op0=ALU.add, op1=ALU.subtract)
            nc.vector.tensor_scalar_max(ur, ur, 0.0)
            r2 = small.tile([128, NT], F32, tag="r2")
            nc.vector.tensor_reduce(r2, ur3, AX.X, ALU.add)
            tript = small.tile([128, NT], F32, tag="tript")
            nc.vector.tensor_add(tript, hard, r2)
            nc.vector.tensor_mul(stat[:, 0:NT], tript, vld_sb)
            # ce = ln(sumexp) - T*s_pos
            lnse = small.tile([128, NT], F32, tag="lnse")
            nc.scalar.activation(lnse, sumexp, AF.Ln)
            tsp = small.tile([128, NT], F32, tag="tsp")
            nc.vector.tensor_scalar_mul(tsp, spos, float(T))
            cet = small.tile([128, NT], F32, tag="cet")
            nc.vector.tensor_sub(cet, lnse, tsp)
            nc.vector.tensor_mul(stat[:, NT:2 * NT], cet, vld_sb)

            # ---- cross-partition reduction: out[j] = sum_p stat[p, j] ----
            pres = psmall.tile([128, 512], F32, tag="pm")
            nc.tensor.matmul(
                pres[:2 * NT, :1], stat, ones_col, start=True, stop=True)
            res_sb = small.tile([2 * NT, 1], F32, tag="res")
            nc.vector.tensor_copy(res_sb, pres[:2 * NT, :1])
            nc.sync.dma_start(out=out[:, :], in_=res_sb[:, :])

    if not nc.is_finalized():
        nc.finalize()
    return nc


_CACHE = {}


def _prep_core(c, cap, pe, ie, bi, mi, ki, rn, T):
    C = NB * cap
    NT = C // 128
    lo = NB * c
    sel = np.where((bi >= lo) & (bi < lo + NB))[0]
    # pad with unit vectors so normalization never divides by zero
    ancb = np.zeros((C, D), np.float32); ancb[:, 0] = 1.0
    posb = np.zeros((C, D), np.float32); posb[:, 0] = 1.0
    rngb = np.zeros((2 * C, D), np.float32); rngb[:, 0] = 1.0
    valid = np.zeros(C, np.float32)
    for n in range(NB):
        pb = sel[bi[sel] == lo + n]
        assert len(pb) <= cap
        s = n * cap
        ancb[s:s + len(pb)] = pe[mi[pb]]
        posb[s:s + len(pb)] = ie[bi[pb], ki[pb]]
        rngb[s:s + len(pb)] = ie[bi[pb], rn[pb, 0]]
        rngb[C + s:C + s + len(pb)] = ie[bi[pb], rn[pb, 1]]
        valid[s:s + len(pb)] = 1.0
    xt_c = np.ascontiguousarray(
        ie[lo:lo + NB].reshape(NB * K, D).T).astype(np.float32)
    vld_dev = np.ascontiguousarray(valid.reshape(NT, 128).T)
    sel4s = np.zeros((128, 16), np.float32)
    for j in range(4):
        sel4s[:, 4 * j + j] = 1.0
    return dict(
        xt=xt_c, phr=pe, anc=ancb, pos=posb, rng=rngb, vld=vld_dev,
        eye=np.eye(128, dtype=np.float32),
        sel4s=sel4s,
    )


def make_in_maps(inputs, cap=None):
    pe = np.asarray(inputs["phrase_embeddings"], np.float32)
    ie = np.asarray(inputs["input_embeddings"], np.float32)
    bi = np.asarray(inputs["batch_idxs"])
    mi = np.asarray(inputs["phrase_emb_idxs"])
    ki = np.asarray(inputs["input_emb_idxs"])
    rn = np.asarray(inputs["rand_neg_idx"])
    T = float(np.asarray(inputs["temperature"]))
    if cap is None:
        maxc = int(np.bincount(bi, minlength=N).max())
        cap = max(64, ((maxc + 63) // 64) * 64)
    return [
        _prep_core(c, cap, pe, ie, bi, mi, ki, rn, T) for c in range(NCORES)
    ], cap, T


def kernel(**inputs):
    in_maps, cap, T = make_in_maps(inputs)
    key = (cap, T)
    if key not in _CACHE:
        _CACHE[key] = build_graph(cap, T)
    nc = _CACHE[key]
    res = run_bass_kernel_spmd(nc, in_maps, core_ids=list(range(NCORES)))
    outs = np.stack([np.asarray(r["out"]).reshape(-1) for r in res.results])
    NT = NB * cap // 128
    trip = outs[:, :NT].sum() / (P * 5)
    ce = outs[:, NT:].sum() / P
    return np.float32(trip), np.float32(ce)
